# revision 22
# baseline (speedup 1.0000x reference)
"""Bayesian multi-task MLP (moe_routing) — Trainium2 Bass/Tile kernel.

Reference computation (per forward):
    w   = mu + exp(ls) * eps                    (Bayesian reparameterization)
    h   = relu(x @ w0.T + b0)                   [4096, 2048]
    h   = relu(h @ wi.T + bi)   for i in 0..2   (2048 -> 2048)
    out = (h @ hw[t].T + hb[t]) routed per-sample by task id   [4096, 10]

Distribution: pure data-parallel over the batch — each of the 8 cores gets
512 rows of x/task and a full replica of the (device-prepared) Bayesian
params.  No collectives.

Host-side prep is layout/dtype only:
  * weight factors transposed to [in, out] so the PE consumes them without
    on-chip transposes (activations stay transposed [hid, batch] throughout);
  * mu / x cast to fp16 (keeps the dominant term at ~5e-4 rel err);
  * ls shipped as (ls + 6) in fp8e4m3 — the shift centers values near 0
    where fp8 has precision, and the device folds the -6 back in via the
    activation bias: t = exp(1.0*d + (-6));
  * eps shipped as fp8e4m3 (it only scales the ~0.25%-magnitude noise term).
This halves-ish HBM traffic (the roofline term) while fp32 PSUM keeps the
scale-relative error ~5e-3.

All math (exp, mul, add, matmuls, bias, relu, routing one-hot select) runs
on-device.  Engine split per weight slab: ACT does exp, GpSimd the t*eps
multiply, DVE the +mu add — three otherwise-idle engines in parallel.
"""

import numpy as np

import concourse.bacc as bacc
import concourse.mybir as mybir
from concourse.bass_utils import run_bass_kernel_spmd
from concourse.tile import TileContext

NCORES = 8
B, IN, H, OUT, T, NL = 4096, 1024, 2048, 10, 10, 3
BC = B // NCORES           # batch rows per core = 512
TO = T * OUT               # flattened head outputs = 100
KT_IN = IN // 128          # k-tiles in layer 0 = 8
KT_H = H // 128            # k-tiles in hidden layers = 16
KG = 4                     # k-tiles per grouped DMA
OT_HALF = 8                # out-tiles per half (8 PSUM banks)

F16 = mybir.dt.float16
F32 = mybir.dt.float32
F8 = mybir.dt.float8e4
ALU = mybir.AluOpType
ACTF = mybir.ActivationFunctionType


def build_nc():
    # Bacc (not raw Bass): its compile() pass legalizes multi-wait sync_infos
    # into EventSemaphore instructions (TRN2 allows 1 wait per instruction).
    nc = bacc.Bacc(trn_type="TRN2")

    # ---- per-core DRAM I/O ----
    xT = nc.dram_tensor("xT", [128, KT_IN, BC], F16, kind="ExternalInput")
    muT0 = nc.dram_tensor("muT0", [IN, H], F16, kind="ExternalInput")
    lsT0 = nc.dram_tensor("lsT0", [IN, H], F8, kind="ExternalInput")
    epsT0 = nc.dram_tensor("epsT0", [IN, H], F8, kind="ExternalInput")
    muT = nc.dram_tensor("muT", [NL, H, H], F16, kind="ExternalInput")
    lsT = nc.dram_tensor("lsT", [NL, H, H], F8, kind="ExternalInput")
    epsT = nc.dram_tensor("epsT", [NL, H, H], F8, kind="ExternalInput")
    # head weights pre-tiled on host to [128, k, to] (contiguous per partition)
    muhT = nc.dram_tensor("muhT", [128, KT_H, TO], F16, kind="ExternalInput")
    lshT = nc.dram_tensor("lshT", [128, KT_H, TO], F16, kind="ExternalInput")
    epshT = nc.dram_tensor("epshT", [128, KT_H, TO], F16, kind="ExternalInput")
    # biases for the 4 dense layers, pre-tiled [128, layer, otile], fp32
    mub = nc.dram_tensor("mub", [128, NL + 1, KT_H], F32, kind="ExternalInput")
    lsb = nc.dram_tensor("lsb", [128, NL + 1, KT_H], F32, kind="ExternalInput")
    epsb = nc.dram_tensor("epsb", [128, NL + 1, KT_H], F32, kind="ExternalInput")
    muhb = nc.dram_tensor("muhb", [1, TO], F32, kind="ExternalInput")
    lshb = nc.dram_tensor("lshb", [1, TO], F32, kind="ExternalInput")
    epshb = nc.dram_tensor("epshb", [1, TO], F32, kind="ExternalInput")
    taskf = nc.dram_tensor("taskf", [128, BC // 128], F32, kind="ExternalInput")
    out = nc.dram_tensor("out", [BC, OUT], F32, kind="ExternalOutput")

    with TileContext(nc) as tc:
        with (
            tc.tile_pool(name="const", bufs=1) as cpool,
            tc.tile_pool(name="wstream", bufs=3) as wpool,
            tc.tile_pool(name="hbuf", bufs=2) as hpool,
            tc.tile_pool(name="sel", bufs=4) as spool,
            tc.tile_pool(name="psum", bufs=8, space="PSUM") as ppool,
        ):
            # ---- dense-layer biases b = mu + exp(ls)*eps  (tiny, fp32) ----
            bias_mu = cpool.tile([128, NL + 1, KT_H], F32)
            bias_ls = cpool.tile([128, NL + 1, KT_H], F32)
            bias_eps = cpool.tile([128, NL + 1, KT_H], F32)
            nc.sync.dma_start(out=bias_mu, in_=mub.ap())
            nc.sync.dma_start(out=bias_ls, in_=lsb.ap())
            nc.sync.dma_start(out=bias_eps, in_=epsb.ap())
            bias = cpool.tile([128, NL + 1, KT_H], F32)
            nc.scalar.activation(out=bias, in_=bias_ls, func=ACTF.Exp)
            nc.vector.tensor_mul(bias, bias, bias_eps)
            nc.vector.tensor_add(bias, bias, bias_mu)

            # per-partition -6.0 for folding the host-side ls shift back out
            neg6 = cpool.tile([128, 1], F32)
            nc.vector.memset(neg6, -6.0)

            # ---- layer 0 input: xT resident in SBUF (split so the first
            # matmuls don't wait on the whole 1MB) ----
            hT_x = hpool.tile([128, KT_IN, BC], F16, tag="hT")
            for xk in range(0, KT_IN, 2):
                nc.sync.dma_start(
                    out=hT_x[:, xk:xk + 2, :], in_=xT.ap()[:, xk:xk + 2, :]
                )

            def ff_layer(hT_in, kt, mu_ap, ls_ap, eps_ap, bias_l, first=False):
                """hT_out[out, b] = relu(w @ hT_in + b); w = mu + exp(ls)*eps.

                mu_ap is the [kt*128, H] fp16 transposed view; ls/eps are the
                fp8 views (ls pre-shifted by +6 on host).
                """
                hT_out = hpool.tile([128, KT_H, BC], F16, tag="hT", name="hT_out")
                for half in range(2):
                    psums = []
                    for o8 in range(OT_HALF):
                        ps = ppool.tile([128, BC], F32, tag="mm", name="ps")
                        psums.append(ps)
                    ocols = slice(half * 1024, (half + 1) * 1024)
                    # small leading groups on the very first half so the first
                    # matmul's DMA dependency is ~0.5MB, not 2MB
                    if first and half == 0:
                        gsizes = [1, 1, 2] + [KG] * ((kt - 4) // KG)
                    else:
                        gsizes = [KG] * (kt // KG)
                    k0 = 0
                    for gs in gsizes:
                        grows = slice(k0 * 128, (k0 + gs) * 128)
                        mu_g = wpool.tile([128, KG, 1024], F16, tag="mu", name="mu_g")
                        ls_g = wpool.tile([128, KG, 1024], F8, tag="ls", name="ls_g")
                        eps_g = wpool.tile([128, KG, 1024], F8, tag="eps", name="eps_g")
                        rearr = lambda ap: ap[grows, ocols].rearrange(
                            "(g p) o -> p g o", p=128
                        )
                        nc.sync.dma_start(out=mu_g[:, :gs, :], in_=rearr(mu_ap))
                        nc.sync.dma_start(out=ls_g[:, :gs, :], in_=rearr(ls_ap))
                        nc.sync.dma_start(out=eps_g[:, :gs, :], in_=rearr(eps_ap))
                        for ks in range(gs):
                            k = k0 + ks
                            t_s = wpool.tile([128, 1024], F16, tag="t", name="t_s")
                            # t = exp(d - 6), d = ls+6 shipped in fp8
                            nc.scalar.activation(
                                out=t_s, in_=ls_g[:, ks, :], func=ACTF.Exp, bias=neg6
                            )
                            # fp8 eps would drop DVE to 1x mode; GpSimd does the
                            # 1-input cast at line rate, DVE stays 2x fp16.
                            eps16 = wpool.tile(
                                [128, 1024], F16, tag="eps16", name="eps16"
                            )
                            nc.gpsimd.tensor_copy(out=eps16, in_=eps_g[:, ks, :])
                            w_s = wpool.tile(
                                [128, 1024], F16, tag="w", bufs=4, name="w_s"
                            )
                            nc.vector.tensor_mul(w_s, t_s, eps16)
                            nc.vector.tensor_add(w_s, w_s, mu_g[:, ks, :])
                            for o8 in range(OT_HALF):
                                nc.tensor.matmul(
                                    psums[o8],
                                    lhsT=w_s[:, o8 * 128:(o8 + 1) * 128],
                                    rhs=hT_in[:, k, :],
                                    start=(k == 0),
                                    stop=(k == kt - 1),
                                )
                        k0 += gs
                    for o8 in range(OT_HALF):
                        o = half * OT_HALF + o8
                        nc.scalar.activation(
                            out=hT_out[:, o, :],
                            in_=psums[o8],
                            func=ACTF.Relu,
                            bias=bias[:, bias_l, o:o + 1],
                        )
                return hT_out

            cur = ff_layer(hT_x, KT_IN, muT0.ap(), lsT0.ap(), epsT0.ap(), 0, first=True)
            for l in range(NL - 1):
                cur = ff_layer(cur, KT_H, muT.ap()[l], lsT.ap()[l], epsT.ap()[l], l + 1)

            # ---- tail constants (issued before the last layer so their DMAs
            # land while its GEMMs run) ----
            hb_mu = cpool.tile([1, TO], F32)
            hb_ls = cpool.tile([1, TO], F32)
            hb_eps = cpool.tile([1, TO], F32)
            nc.sync.dma_start(out=hb_mu, in_=muhb.ap())
            nc.sync.dma_start(out=hb_ls, in_=lshb.ap())
            nc.sync.dma_start(out=hb_eps, in_=epshb.ap())
            hb_f = cpool.tile([1, TO], F32)
            nc.scalar.activation(out=hb_f, in_=hb_ls, func=ACTF.Exp)
            nc.vector.tensor_mul(hb_f, hb_f, hb_eps)
            nc.vector.tensor_add(hb_f, hb_f, hb_mu)
            hb16 = cpool.tile([1, TO], F16)
            nc.vector.tensor_copy(out=hb16, in_=hb_f)
            ones1 = cpool.tile([1, 128], F16)
            nc.vector.memset(ones1, 1.0)

            # head weights: whT[p, k, to] = mu + exp(ls)*eps  (fp16, ~1.2MB)
            wh_mu = cpool.tile([128, KT_H, TO], F16)
            wh_ls = cpool.tile([128, KT_H, TO], F16)
            wh_eps = cpool.tile([128, KT_H, TO], F16)
            nc.sync.dma_start(out=wh_mu, in_=muhT.ap())
            nc.sync.dma_start(out=wh_ls, in_=lshT.ap())
            nc.sync.dma_start(out=wh_eps, in_=epshT.ap())
            whT = cpool.tile([128, KT_H, TO], F16)
            nc.scalar.activation(out=whT, in_=wh_ls, func=ACTF.Exp)
            nc.vector.tensor_mul(whT, whT, wh_eps)
            nc.vector.tensor_add(whT, whT, wh_mu)

            taskt = cpool.tile([128, BC // 128], F32)
            nc.sync.dma_start(out=taskt, in_=taskf.ap())
            iota10 = cpool.tile([128, T], mybir.dt.int32)
            nc.gpsimd.iota(iota10, [[1, T]], base=0, channel_multiplier=0)
            iota10f = cpool.tile([128, T], F32)
            nc.vector.tensor_copy(out=iota10f, in_=iota10)

            # ---- last hidden layer ----
            cur = ff_layer(
                cur, KT_H, muT.ap()[NL - 1], lsT.ap()[NL - 1], epsT.ap()[NL - 1], NL
            )

            # ---- heads + routing select ----
            for m in range(BC // 128):
                ps = ppool.tile([128, TO], F32, tag="mm", name="ps_head")
                for k in range(KT_H):
                    nc.tensor.matmul(
                        ps,
                        lhsT=cur[:, k, m * 128:(m + 1) * 128],
                        rhs=whT[:, k, :],
                        start=(k == 0),
                        stop=False,
                    )
                nc.tensor.matmul(
                    ps, lhsT=ones1[:1, :], rhs=hb16[:1, :], start=False, stop=True
                )
                onehot = spool.tile([128, T], F32, name="onehot")
                nc.vector.tensor_single_scalar(
                    out=onehot, in_=iota10f, scalar=taskt[:, m:m + 1], op=ALU.is_equal
                )
                masked = spool.tile([128, OUT, T], F32, name="masked")
                ps_v = ps.rearrange("p (t o) -> p o t", t=T)
                oh_v = onehot.unsqueeze(1).broadcast_to([128, OUT, T])
                nc.vector.tensor_tensor(masked, ps_v, oh_v, ALU.mult)
                outm = spool.tile([128, OUT], F32, name="outm")
                nc.vector.tensor_reduce(
                    out=outm, in_=masked, axis=mybir.AxisListType.X, op=ALU.add
                )
                nc.sync.dma_start(out=out.ap()[m * 128:(m + 1) * 128, :], in_=outm)

    nc.finalize()
    return nc


_CACHE = {}


def _prep_host(inputs):
    """Layout/dtype prep + batch sharding. Returns list of per-core in_maps."""
    import ml_dtypes

    f16 = np.float16
    f8 = ml_dtypes.float8_e4m3fn

    def bias_tile(b0, b):  # [4, H] -> [128, 4, 16]
        arr = np.concatenate([b0[None], b], 0).astype(np.float32)
        return np.ascontiguousarray(arr.reshape(NL + 1, KT_H, 128).transpose(2, 0, 1))

    def head_tile(a):  # [T, OUT, H] -> headT [H, TO] -> [128, 16, TO]
        aT = a.reshape(TO, H).astype(f16).T
        return np.ascontiguousarray(aT.reshape(KT_H, 128, TO).transpose(1, 0, 2))

    shared = {
        "muT0": np.ascontiguousarray(inputs["mu_w0"].astype(f16).T),
        "lsT0": np.ascontiguousarray((inputs["ls_w0"].T + 6.0).astype(f8)),
        "epsT0": np.ascontiguousarray(inputs["eps_w0"].T.astype(f8)),
        "muT": np.ascontiguousarray(inputs["mu_w"].astype(f16).transpose(0, 2, 1)),
        "lsT": np.ascontiguousarray((inputs["ls_w"].transpose(0, 2, 1) + 6.0).astype(f8)),
        "epsT": np.ascontiguousarray(inputs["eps_w"].transpose(0, 2, 1).astype(f8)),
        "muhT": head_tile(inputs["mu_hw"]),
        "lshT": head_tile(inputs["ls_hw"]),
        "epshT": head_tile(inputs["eps_hw"]),
        "mub": bias_tile(inputs["mu_b0"], inputs["mu_b"]),
        "lsb": bias_tile(inputs["ls_b0"], inputs["ls_b"]),
        "epsb": bias_tile(inputs["eps_b0"], inputs["eps_b"]),
        "muhb": inputs["mu_hb"].reshape(1, TO).astype(np.float32),
        "lshb": inputs["ls_hb"].reshape(1, TO).astype(np.float32),
        "epshb": inputs["eps_hb"].reshape(1, TO).astype(np.float32),
    }
    xT = inputs["x"].astype(f16).T  # [IN, B]
    task = inputs["task"].astype(np.float32)
    in_maps = []
    for c in range(NCORES):
        m = dict(shared)
        xc = xT[:, c * BC:(c + 1) * BC]  # [IN, BC]
        m["xT"] = np.ascontiguousarray(xc.reshape(KT_IN, 128, BC).transpose(1, 0, 2))
        m["taskf"] = np.ascontiguousarray(
            task[c * BC:(c + 1) * BC].reshape(BC // 128, 128).T
        )
        in_maps.append(m)
    return in_maps


def kernel(**inputs):
    inputs = {k: np.asarray(v) for k, v in inputs.items()}
    if "nc" not in _CACHE:
        _CACHE["nc"] = build_nc()
    nc = _CACHE["nc"]
    in_maps = _prep_host(inputs)
    res = run_bass_kernel_spmd(nc, in_maps, core_ids=list(range(NCORES)))
    out = np.concatenate([res.results[c]["out"] for c in range(NCORES)], axis=0)
    return out.astype(np.float32)


if __name__ == "__main__":
    nc = build_nc()
    print("built ok")


# revision 24
# speedup vs baseline: 2.0840x; 2.0840x over previous
"""Bayesian multi-task MLP (moe_routing) — Trainium2 Bass/Tile kernel.

Reference computation (per forward):
    w   = mu + exp(ls) * eps                    (Bayesian reparameterization)
    h   = relu(x @ w0.T + b0)                   [4096, 2048]
    h   = relu(h @ wi.T + bi)   for i in 0..2   (2048 -> 2048)
    out = (h @ hw[t].T + hb[t]) routed per-sample by task id   [4096, 10]

Distribution: pure data-parallel over the batch — each of the 8 cores gets
512 rows of x/task and a full replica of the (device-prepared) Bayesian
params.  No collectives.

Host-side prep is layout/dtype only:
  * weight factors transposed to [in, out] so the PE consumes them without
    on-chip transposes (activations stay transposed [hid, batch] throughout);
  * mu / x cast to fp16 (keeps the dominant term at ~5e-4 rel err);
  * ls shipped as (ls + 6) in fp8e4m3 — the shift centers values near 0
    where fp8 has precision, and the device folds the -6 back in via the
    activation bias: t = exp(1.0*d + (-6));
  * eps shipped as fp8e4m3 (it only scales the ~0.25%-magnitude noise term).
This halves-ish HBM traffic (the roofline term) while fp32 PSUM keeps the
scale-relative error ~5e-3.

All math (exp, mul, add, matmuls, bias, relu, routing one-hot select) runs
on-device.  Engine split per weight slab: ACT does exp, GpSimd the t*eps
multiply, DVE the +mu add — three otherwise-idle engines in parallel.
"""

import numpy as np

import concourse.bacc as bacc
import concourse.mybir as mybir
from concourse.bass_utils import run_bass_kernel_spmd
from concourse.tile import TileContext

NCORES = 8
B, IN, H, OUT, T, NL = 4096, 1024, 2048, 10, 10, 3
BC = B // NCORES           # batch rows per core = 512
TO = T * OUT               # flattened head outputs = 100
KT_IN = IN // 128          # k-tiles in layer 0 = 8
KT_H = H // 128            # k-tiles in hidden layers = 16
KG = 4                     # k-tiles per grouped DMA
OT_HALF = 8                # out-tiles per half (8 PSUM banks)

F16 = mybir.dt.float16
F32 = mybir.dt.float32
F8 = mybir.dt.float8e4
ALU = mybir.AluOpType
ACTF = mybir.ActivationFunctionType


def build_nc():
    # Bacc (not raw Bass): its compile() pass legalizes multi-wait sync_infos
    # into EventSemaphore instructions (TRN2 allows 1 wait per instruction).
    nc = bacc.Bacc(trn_type="TRN2")

    # ---- per-core DRAM I/O ----
    xT = nc.dram_tensor("xT", [128, KT_IN, BC], F16, kind="ExternalInput")
    muT0 = nc.dram_tensor("muT0", [IN, H], F16, kind="ExternalInput")
    lsT0 = nc.dram_tensor("lsT0", [IN, H], F8, kind="ExternalInput")
    epsT0 = nc.dram_tensor("epsT0", [IN, H], F16, kind="ExternalInput")
    muT = nc.dram_tensor("muT", [NL, H, H], F16, kind="ExternalInput")
    lsT = nc.dram_tensor("lsT", [NL, H, H], F8, kind="ExternalInput")
    epsT = nc.dram_tensor("epsT", [NL, H, H], F16, kind="ExternalInput")
    # head weights pre-tiled on host to [128, k, to] (contiguous per partition)
    muhT = nc.dram_tensor("muhT", [128, KT_H, TO], F16, kind="ExternalInput")
    lshT = nc.dram_tensor("lshT", [128, KT_H, TO], F16, kind="ExternalInput")
    epshT = nc.dram_tensor("epshT", [128, KT_H, TO], F16, kind="ExternalInput")
    # biases for the 4 dense layers, pre-tiled [128, layer, otile], fp32
    mub = nc.dram_tensor("mub", [128, NL + 1, KT_H], F32, kind="ExternalInput")
    lsb = nc.dram_tensor("lsb", [128, NL + 1, KT_H], F32, kind="ExternalInput")
    epsb = nc.dram_tensor("epsb", [128, NL + 1, KT_H], F32, kind="ExternalInput")
    muhb = nc.dram_tensor("muhb", [1, TO], F32, kind="ExternalInput")
    lshb = nc.dram_tensor("lshb", [1, TO], F32, kind="ExternalInput")
    epshb = nc.dram_tensor("epshb", [1, TO], F32, kind="ExternalInput")
    taskf = nc.dram_tensor("taskf", [128, BC // 128], F32, kind="ExternalInput")
    out = nc.dram_tensor("out", [BC, OUT], F32, kind="ExternalOutput")

    with TileContext(nc) as tc:
        with (
            tc.tile_pool(name="const", bufs=1) as cpool,
            tc.tile_pool(name="wstream", bufs=3) as wpool,
            tc.tile_pool(name="hbuf", bufs=2) as hpool,
            tc.tile_pool(name="sel", bufs=4) as spool,
            tc.tile_pool(name="psum", bufs=8, space="PSUM") as ppool,
        ):
            # ---- dense-layer biases b = mu + exp(ls)*eps  (tiny, fp32) ----
            bias_mu = cpool.tile([128, NL + 1, KT_H], F32)
            bias_ls = cpool.tile([128, NL + 1, KT_H], F32)
            bias_eps = cpool.tile([128, NL + 1, KT_H], F32)
            nc.sync.dma_start(out=bias_mu, in_=mub.ap())
            nc.sync.dma_start(out=bias_ls, in_=lsb.ap())
            nc.sync.dma_start(out=bias_eps, in_=epsb.ap())
            bias = cpool.tile([128, NL + 1, KT_H], F32)
            nc.scalar.activation(out=bias, in_=bias_ls, func=ACTF.Exp)
            nc.vector.tensor_mul(bias, bias, bias_eps)
            nc.vector.tensor_add(bias, bias, bias_mu)

            # per-partition -6.0 for folding the host-side ls shift back out
            neg6 = cpool.tile([128, 1], F32)
            nc.vector.memset(neg6, -6.0)

            # ---- layer 0 input: xT resident in SBUF (split so the first
            # matmuls don't wait on the whole 1MB) ----
            hT_x = hpool.tile([128, KT_IN, BC], F16, tag="hT")
            for xk in range(0, KT_IN, 2):
                nc.sync.dma_start(
                    out=hT_x[:, xk:xk + 2, :], in_=xT.ap()[:, xk:xk + 2, :]
                )

            def ff_layer(hT_in, kt, mu_ap, ls_ap, eps_ap, bias_l, first=False):
                """hT_out[out, b] = relu(w @ hT_in + b); w = mu + exp(ls)*eps.

                mu_ap is the [kt*128, H] fp16 transposed view; ls/eps are the
                fp8 views (ls pre-shifted by +6 on host).
                """
                hT_out = hpool.tile([128, KT_H, BC], F16, tag="hT", name="hT_out")
                for half in range(2):
                    psums = []
                    for o8 in range(OT_HALF):
                        ps = ppool.tile([128, BC], F32, tag="mm", name="ps")
                        psums.append(ps)
                    ocols = slice(half * 1024, (half + 1) * 1024)
                    # small leading groups on the very first half so the first
                    # matmul's DMA dependency is ~0.5MB, not 2MB
                    if first and half == 0:
                        gsizes = [1, 1, 2] + [KG] * ((kt - 4) // KG)
                    else:
                        gsizes = [KG] * (kt // KG)
                    k0 = 0
                    for gs in gsizes:
                        grows = slice(k0 * 128, (k0 + gs) * 128)
                        mu_g = wpool.tile([128, KG, 1024], F16, tag="mu", name="mu_g")
                        ls_g = wpool.tile([128, KG, 1024], F8, tag="ls", name="ls_g")
                        eps_g = wpool.tile([128, KG, 1024], F16, tag="eps", name="eps_g")
                        rearr = lambda ap: ap[grows, ocols].rearrange(
                            "(g p) o -> p g o", p=128
                        )
                        nc.sync.dma_start(out=mu_g[:, :gs, :], in_=rearr(mu_ap))
                        nc.sync.dma_start(out=ls_g[:, :gs, :], in_=rearr(ls_ap))
                        nc.sync.dma_start(out=eps_g[:, :gs, :], in_=rearr(eps_ap))
                        for ks in range(gs):
                            k = k0 + ks
                            t_s = wpool.tile([128, 1024], F16, tag="t", name="t_s")
                            # t = exp(d - 6), d = ls+6 shipped in fp8
                            nc.scalar.activation(
                                out=t_s, in_=ls_g[:, ks, :], func=ACTF.Exp, bias=neg6
                            )
                            w_s = wpool.tile(
                                [128, 1024], F16, tag="w", bufs=4, name="w_s"
                            )
                            nc.vector.tensor_mul(w_s, t_s, eps_g[:, ks, :])
                            nc.vector.tensor_add(w_s, w_s, mu_g[:, ks, :])
                            for o8 in range(OT_HALF):
                                nc.tensor.matmul(
                                    psums[o8],
                                    lhsT=w_s[:, o8 * 128:(o8 + 1) * 128],
                                    rhs=hT_in[:, k, :],
                                    start=(k == 0),
                                    stop=(k == kt - 1),
                                )
                        k0 += gs
                    for o8 in range(OT_HALF):
                        o = half * OT_HALF + o8
                        nc.scalar.activation(
                            out=hT_out[:, o, :],
                            in_=psums[o8],
                            func=ACTF.Relu,
                            bias=bias[:, bias_l, o:o + 1],
                        )
                return hT_out

            cur = ff_layer(hT_x, KT_IN, muT0.ap(), lsT0.ap(), epsT0.ap(), 0, first=True)
            for l in range(NL - 1):
                cur = ff_layer(cur, KT_H, muT.ap()[l], lsT.ap()[l], epsT.ap()[l], l + 1)

            # ---- tail constants (issued before the last layer so their DMAs
            # land while its GEMMs run) ----
            hb_mu = cpool.tile([1, TO], F32)
            hb_ls = cpool.tile([1, TO], F32)
            hb_eps = cpool.tile([1, TO], F32)
            nc.sync.dma_start(out=hb_mu, in_=muhb.ap())
            nc.sync.dma_start(out=hb_ls, in_=lshb.ap())
            nc.sync.dma_start(out=hb_eps, in_=epshb.ap())
            hb_f = cpool.tile([1, TO], F32)
            nc.scalar.activation(out=hb_f, in_=hb_ls, func=ACTF.Exp)
            nc.vector.tensor_mul(hb_f, hb_f, hb_eps)
            nc.vector.tensor_add(hb_f, hb_f, hb_mu)
            hb16 = cpool.tile([1, TO], F16)
            nc.vector.tensor_copy(out=hb16, in_=hb_f)
            ones1 = cpool.tile([1, 128], F16)
            nc.vector.memset(ones1, 1.0)

            # head weights: whT[p, k, to] = mu + exp(ls)*eps  (fp16, ~1.2MB)
            wh_mu = cpool.tile([128, KT_H, TO], F16)
            wh_ls = cpool.tile([128, KT_H, TO], F16)
            wh_eps = cpool.tile([128, KT_H, TO], F16)
            nc.sync.dma_start(out=wh_mu, in_=muhT.ap())
            nc.sync.dma_start(out=wh_ls, in_=lshT.ap())
            nc.sync.dma_start(out=wh_eps, in_=epshT.ap())
            whT = cpool.tile([128, KT_H, TO], F16)
            nc.scalar.activation(out=whT, in_=wh_ls, func=ACTF.Exp)
            nc.vector.tensor_mul(whT, whT, wh_eps)
            nc.vector.tensor_add(whT, whT, wh_mu)

            taskt = cpool.tile([128, BC // 128], F32)
            nc.sync.dma_start(out=taskt, in_=taskf.ap())
            iota10 = cpool.tile([128, T], mybir.dt.int32)
            nc.gpsimd.iota(iota10, [[1, T]], base=0, channel_multiplier=0)
            iota10f = cpool.tile([128, T], F32)
            nc.vector.tensor_copy(out=iota10f, in_=iota10)

            # ---- last hidden layer ----
            cur = ff_layer(
                cur, KT_H, muT.ap()[NL - 1], lsT.ap()[NL - 1], epsT.ap()[NL - 1], NL
            )

            # ---- heads + routing select ----
            for m in range(BC // 128):
                ps = ppool.tile([128, TO], F32, tag="mm", name="ps_head")
                for k in range(KT_H):
                    nc.tensor.matmul(
                        ps,
                        lhsT=cur[:, k, m * 128:(m + 1) * 128],
                        rhs=whT[:, k, :],
                        start=(k == 0),
                        stop=False,
                    )
                nc.tensor.matmul(
                    ps, lhsT=ones1[:1, :], rhs=hb16[:1, :], start=False, stop=True
                )
                onehot = spool.tile([128, T], F32, name="onehot")
                nc.vector.tensor_single_scalar(
                    out=onehot, in_=iota10f, scalar=taskt[:, m:m + 1], op=ALU.is_equal
                )
                masked = spool.tile([128, OUT, T], F32, name="masked")
                ps_v = ps.rearrange("p (t o) -> p o t", t=T)
                oh_v = onehot.unsqueeze(1).broadcast_to([128, OUT, T])
                nc.vector.tensor_tensor(masked, ps_v, oh_v, ALU.mult)
                outm = spool.tile([128, OUT], F32, name="outm")
                nc.vector.tensor_reduce(
                    out=outm, in_=masked, axis=mybir.AxisListType.X, op=ALU.add
                )
                nc.sync.dma_start(out=out.ap()[m * 128:(m + 1) * 128, :], in_=outm)

    nc.finalize()
    return nc


_CACHE = {}


def _prep_host(inputs):
    """Layout/dtype prep + batch sharding. Returns list of per-core in_maps."""
    import ml_dtypes

    f16 = np.float16
    f8 = ml_dtypes.float8_e4m3fn

    def bias_tile(b0, b):  # [4, H] -> [128, 4, 16]
        arr = np.concatenate([b0[None], b], 0).astype(np.float32)
        return np.ascontiguousarray(arr.reshape(NL + 1, KT_H, 128).transpose(2, 0, 1))

    def head_tile(a):  # [T, OUT, H] -> headT [H, TO] -> [128, 16, TO]
        aT = a.reshape(TO, H).astype(f16).T
        return np.ascontiguousarray(aT.reshape(KT_H, 128, TO).transpose(1, 0, 2))

    shared = {
        "muT0": np.ascontiguousarray(inputs["mu_w0"].astype(f16).T),
        "lsT0": np.ascontiguousarray((inputs["ls_w0"].T + 6.0).astype(f8)),
        "epsT0": np.ascontiguousarray(inputs["eps_w0"].astype(f16).T),
        "muT": np.ascontiguousarray(inputs["mu_w"].astype(f16).transpose(0, 2, 1)),
        "lsT": np.ascontiguousarray((inputs["ls_w"].transpose(0, 2, 1) + 6.0).astype(f8)),
        "epsT": np.ascontiguousarray(inputs["eps_w"].astype(f16).transpose(0, 2, 1)),
        "muhT": head_tile(inputs["mu_hw"]),
        "lshT": head_tile(inputs["ls_hw"]),
        "epshT": head_tile(inputs["eps_hw"]),
        "mub": bias_tile(inputs["mu_b0"], inputs["mu_b"]),
        "lsb": bias_tile(inputs["ls_b0"], inputs["ls_b"]),
        "epsb": bias_tile(inputs["eps_b0"], inputs["eps_b"]),
        "muhb": inputs["mu_hb"].reshape(1, TO).astype(np.float32),
        "lshb": inputs["ls_hb"].reshape(1, TO).astype(np.float32),
        "epshb": inputs["eps_hb"].reshape(1, TO).astype(np.float32),
    }
    xT = inputs["x"].astype(f16).T  # [IN, B]
    task = inputs["task"].astype(np.float32)
    in_maps = []
    for c in range(NCORES):
        m = dict(shared)
        xc = xT[:, c * BC:(c + 1) * BC]  # [IN, BC]
        m["xT"] = np.ascontiguousarray(xc.reshape(KT_IN, 128, BC).transpose(1, 0, 2))
        m["taskf"] = np.ascontiguousarray(
            task[c * BC:(c + 1) * BC].reshape(BC // 128, 128).T
        )
        in_maps.append(m)
    return in_maps


def kernel(**inputs):
    inputs = {k: np.asarray(v) for k, v in inputs.items()}
    if "nc" not in _CACHE:
        _CACHE["nc"] = build_nc()
    nc = _CACHE["nc"]
    in_maps = _prep_host(inputs)
    res = run_bass_kernel_spmd(nc, in_maps, core_ids=list(range(NCORES)))
    out = np.concatenate([res.results[c]["out"] for c in range(NCORES)], axis=0)
    return out.astype(np.float32)


if __name__ == "__main__":
    nc = build_nc()
    print("built ok")
